# revision 7
# baseline (speedup 1.0000x reference)
"""Multi-head attention kernel for 8 Trainium2 NeuronCores — v2.

Problem: B=4, L=2048, DIM=1024, H=16 heads, d_k=d_v=64.
  qh = q @ Wq_h ; kh = k @ Wk_h ; vh = v @ Wv_h          (per head)
  out_h = softmax(qh kh^T / sqrt(DIM)) vh
  y = concat_h(out_h) @ proj_w.T + proj_b

Sharding: data-parallel over (batch, query-half): core c handles batch
c//2, query rows [1024*(c%2), ...+1024). K/V projections are computed
half per core and merged with a core-pair AllGather.

v2 design (vs baseline):
- Scores are row-tiled K=64 matmuls: the head-pair's (even, odd)
  stationaries live at PE-array rows 0:64 / 64:128 (tile_position
  auto-derived from base_partition) and run concurrently — no
  zero-padded K=128 waste.
- The scalar engine's exp (256 x [128,1024], ~293us) is the bottleneck;
  everything else is scheduled into its shadow: V/K/Q projection units
  are emitted inside the attention pair-loop, qh1 PV chains are
  deferred to per-pair bursts so only 8 PSUM banks are needed:
  scores_e 2 + scores_o 2 + pv(qh0/V-proj) 2 + aux(proj/qh1) 2.
- Dense output projection: oall2[hp] is [128 rows = head pair, 1024 q]
  (odd head's normalized PV result is DMA partition-shifted to rows
  64:128), so phase C contracts 8 dense 128-row chunks.
- Softmax denominators ride as a ones-column in the PV stationary
  (M=65); reciprocal via the fast custom-DVE approx; broadcast across
  partitions via a DRAM bounce.
"""

import numpy as np

P = 128
B, L, DIM, H, DK = 4, 2048, 1024, 16, 64
TQ = 1024      # q tokens per core
TS = 2048      # kv tokens per core
NDCH = DIM // P          # 8 contraction chunks
NHP = H // 2             # 8 head pairs
NST = TS // P            # 16 key tiles
N_CORES = 8

_NC = None
TRACE = False
LAST_RESULT = None


def _build():
    from contextlib import ExitStack

    import concourse.bass as bass
    from concourse import bacc
    import concourse.mybir as mybir
    import concourse.tile as tile

    DT_B = mybir.dt.bfloat16
    DT_F = mybir.dt.float32
    AF = mybir.ActivationFunctionType

    nc = bacc.Bacc(None, target_bir_lowering=False)
    qT = nc.dram_tensor("qT", [DIM, TQ], DT_B, kind="ExternalInput")
    kT = nc.dram_tensor("kTh", [DIM, TS // 2], DT_B, kind="ExternalInput")
    vT = nc.dram_tensor("vTh", [DIM, TS // 2], DT_B, kind="ExternalInput")
    wq = nc.dram_tensor("wq", [DIM, H * DK], DT_B, kind="ExternalInput")
    wk = nc.dram_tensor("wk", [DIM, H * DK], DT_B, kind="ExternalInput")
    wv = nc.dram_tensor("wv", [DIM, H * DK], DT_B, kind="ExternalInput")
    pw = nc.dram_tensor("pwT", [H * DK, DIM], DT_B, kind="ExternalInput")
    pb = nc.dram_tensor("pb", [P, NDCH], DT_F, kind="ExternalInput")
    yT = nc.dram_tensor("yT", [DIM, TQ], DT_F, kind="ExternalOutput")

    def bcast_ap(ap, count):
        return bass.AP(tensor=ap.tensor, offset=ap.offset,
                       ap=[[0, count]] + [list(x) for x in ap.ap[1:]])

    with tile.TileContext(nc) as tc, ExitStack() as es:
        # ---------------- static SBUF pools ----------------
        l1 = es.enter_context(tc.tile_pool(name="l1", bufs=1))
        expp = es.enter_context(tc.tile_pool(name="expp", bufs=27))
        khtp = es.enter_context(tc.tile_pool(name="khtp", bufs=2))
        qhtp = es.enter_context(tc.tile_pool(name="qhtp", bufs=2))
        wqp = es.enter_context(tc.tile_pool(name="wqp", bufs=2))
        wkp = es.enter_context(tc.tile_pool(name="wkp", bufs=2))
        stg = es.enter_context(tc.tile_pool(name="stg", bufs=2))   # proj staging
        smp = es.enter_context(tc.tile_pool(name="smp", bufs=2))   # recip rows
        bcp = es.enter_context(tc.tile_pool(name="bcp", bufs=2))   # broadcasts
        otp = es.enter_context(tc.tile_pool(name="otp", bufs=2))   # odd-head tmp
        # ---------------- PSUM pools: 2+2+2+2 = 8 banks ----------------
        pse = es.enter_context(tc.tile_pool(name="pse", bufs=1, space="PSUM"))
        pso = es.enter_context(tc.tile_pool(name="pso", bufs=1, space="PSUM"))
        ppv = es.enter_context(tc.tile_pool(name="ppv", bufs=2, space="PSUM"))
        paux = es.enter_context(tc.tile_pool(name="paux", bufs=2, space="PSUM"))
        # ---------------- DRAM pools ----------------
        drp = es.enter_context(tc.tile_pool(name="drp", bufs=1, space="DRAM"))
        bncp = es.enter_context(tc.tile_pool(name="bncp", bufs=8, space="DRAM"))
        # input pools, released mid-emission once consumed (LIFO order)
        kinp = tc.alloc_tile_pool(name="kinp", bufs=1)
        qinp = tc.alloc_tile_pool(name="qinp", bufs=1)
        vinp = tc.alloc_tile_pool(name="vinp", bufs=1)

        # ---------------- whole-kernel tiles ----------------
        vhp = [l1.tile([P, H, DK + 1], DT_B, name=f"vhp{i}") for i in range(NST)]
        oall2 = [l1.tile([P, TQ], DT_B, name=f"oall2_{i}") for i in range(NHP)]
        pbt = l1.tile([P, NDCH], DT_F, name="pbt")
        nc.sync.dma_start(out=pbt[:, :], in_=pb[:, :])

        # ---------------- input DMAs ----------------
        vin = [vinp.tile([P, TS // 2], DT_B, name=f"vin{d}") for d in range(NDCH)]
        wvt = [vinp.tile([P, H * DK], DT_B, name=f"wvt{d}") for d in range(NDCH)]
        kin = [kinp.tile([P, TS // 2], DT_B, name=f"kin{d}") for d in range(NDCH)]
        qin = [qinp.tile([P, TQ], DT_B, name=f"qin{d}") for d in range(NDCH)]
        for d in range(NDCH):
            sl = slice(d * P, (d + 1) * P)
            nc.scalar.dma_start(out=vin[d][:, :], in_=vT[sl, :])
            nc.scalar.dma_start(out=wvt[d][:, :], in_=wv[sl, :])
            nc.sync.dma_start(out=kin[d][:, :], in_=kT[sl, :])
            nc.gpsimd.dma_start(out=qin[d][:, :], in_=qT[sl, :])

        # DRAM staging for collectives
        kout = [drp.tile([P, TS // 2], DT_B, name=f"kout{hp}")
                for hp in range(NHP)]
        kgath = [drp.tile([2, P, TS // 2], DT_B, name=f"kgath{hp}")
                 for hp in range(NHP)]
        vout = drp.tile([NST // 2, P, H * (DK + 1)], DT_B, name="vout")
        vgath = drp.tile([2, NST // 2, P, H * (DK + 1)], DT_B, name="vgath")

        kht = [None] * NHP
        qht = [None] * NHP

        # ---------------- projection units ----------------
        def k_unit(hp):
            """Project this core's key-half for head pair hp, AllGather."""
            wsl = wkp.tile([P, NDCH, P], DT_B, name=f"wksl{hp}", tag="wk")
            nc.sync.dma_start(
                out=wsl[:, :, :],
                in_=wk[:, hp * P:(hp + 1) * P].rearrange("(c p) n -> p c n", p=P))
            ps = [paux.tile([P, 512], DT_F, name=f"kps_{hp}_{n}", tag="aux")
                  for n in range(2)]
            for d in range(NDCH):
                for n in range(2):
                    nc.tensor.matmul(ps[n][:, :],
                                     wsl[:, d, :],
                                     kin[d][:, n * 512:(n + 1) * 512],
                                     start=(d == 0), stop=(d == NDCH - 1))
            kst = stg.tile([P, TS // 2], DT_B, name=f"kst{hp}", tag="kst")
            for n in range(2):
                nc.vector.tensor_copy(kst[:, n * 512:(n + 1) * 512], ps[n][:, :])
            nc.sync.dma_start(out=kout[hp][:, :], in_=kst[:, :])
            nc.gpsimd.collective_compute(
                "AllGather", mybir.AluOpType.bypass,
                replica_groups=[[0, 1], [2, 3], [4, 5], [6, 7]],
                ins=[kout[hp][:, :]], outs=[kgath[hp][:, :, :]])
            kht[hp] = khtp.tile([P, TS], DT_B, name=f"kht{hp}", tag="kht")
            for half in range(2):
                nc.gpsimd.dma_start(
                    out=kht[hp][:, half * (TS // 2):(half + 1) * (TS // 2)],
                    in_=kgath[hp][half, :, :])

        def q_unit(hp):
            wsl = wqp.tile([P, NDCH, P], DT_B, name=f"wqsl{hp}", tag="wq")
            nc.sync.dma_start(
                out=wsl[:, :, :],
                in_=wq[:, hp * P:(hp + 1) * P].rearrange("(c p) n -> p c n", p=P))
            ps = [paux.tile([P, 512], DT_F, name=f"qps_{hp}_{n}", tag="aux")
                  for n in range(2)]
            for d in range(NDCH):
                for n in range(2):
                    nc.tensor.matmul(ps[n][:, :],
                                     wsl[:, d, :],
                                     qin[d][:, n * 512:(n + 1) * 512],
                                     start=(d == 0), stop=(d == NDCH - 1))
            qht[hp] = qhtp.tile([P, TQ], DT_B, name=f"qht{hp}", tag="qht")
            for n in range(2):
                nc.vector.tensor_copy(qht[hp][:, n * 512:(n + 1) * 512],
                                      ps[n][:, :])

        def v_unit(st):
            """Project this core's value-half for key tile st (of 8)."""
            ps0 = ppv.tile([P, 512], DT_F, name=f"vps_{st}_0", tag="pv")
            ps1 = paux.tile([P, 512], DT_F, name=f"vps_{st}_1", tag="aux")
            ps = [ps0, ps1]
            for d in range(NDCH):
                for n in range(2):
                    nc.tensor.matmul(ps[n][:, :],
                                     vin[d][:, st * P:(st + 1) * P],
                                     wvt[d][:, n * 512:(n + 1) * 512],
                                     start=(d == 0), stop=(d == NDCH - 1))
            vst = stg.tile([P, H, DK + 1], DT_B, name=f"vst{st}", tag="vst")
            for n in range(2):
                nc.vector.tensor_copy(
                    vst[:, n * 8:(n + 1) * 8, 0:DK],
                    ps[n][:, :].rearrange("p (h d) -> p h d", d=DK))
            nc.vector.memset(vst[:, :, DK:DK + 1], 1.0)
            nc.sync.dma_start(out=vout[st, :, :],
                              in_=vst[:, :, :].rearrange("p h d -> p (h d)"))

        def v_gather():
            nc.gpsimd.collective_compute(
                "AllGather", mybir.AluOpType.bypass,
                replica_groups=[[0, 1], [2, 3], [4, 5], [6, 7]],
                ins=[vout[:, :, :]], outs=[vgath[:, :, :, :]])
            for half in range(2):
                for st in range(NST // 2):
                    nc.gpsimd.dma_start(
                        out=vhp[half * (NST // 2) + st][:, :, :],
                        in_=vgath[half, st, :, :].rearrange(
                            "p (h d) -> p h d", d=DK + 1))

        # ---------------- phase B pieces ----------------
        exp_tiles = [[None] * NST for _ in range(H)]

        def scores_exp(p, kt):
            sp_e = pse.tile([P, TQ], DT_F, name=f"se_{p}_{kt}", tag="se")
            sp_o = pso.tile([P, TQ], DT_F, name=f"so_{p}_{kt}", tag="so")
            for n in range(2):
                nc.tensor.matmul(
                    sp_e[:, n * 512:(n + 1) * 512],
                    kht[p][0:DK, kt * P:(kt + 1) * P],
                    qht[p][0:DK, n * 512:(n + 1) * 512],
                    start=True, stop=True)
                nc.tensor.matmul(
                    sp_o[:, n * 512:(n + 1) * 512],
                    kht[p][DK:P, kt * P:(kt + 1) * P],
                    qht[p][DK:P, n * 512:(n + 1) * 512],
                    start=True, stop=True)
            ex_e = expp.tile([P, TQ], DT_B, name=f"exe_{p}_{kt}", tag="exp")
            ex_o = expp.tile([P, TQ], DT_B, name=f"exo_{p}_{kt}", tag="exp")
            nc.scalar.activation(ex_e[:, :], sp_e[:, :], AF.Exp, scale=1.0 / 32.0)
            nc.scalar.activation(ex_o[:, :], sp_o[:, :], AF.Exp, scale=1.0 / 32.0)
            exp_tiles[2 * p][kt] = ex_e
            exp_tiles[2 * p + 1][kt] = ex_o

        pv_ps = {}   # (head, qh) -> psum tile of the running chain

        def pv_chain_mm(h, qh, kt, pool):
            if kt == 0:
                tag = "pv" if pool is ppv else "aux"
                pv_ps[(h, qh)] = pool.tile([P, 512], DT_F,
                                           name=f"pv_{h}_{qh}", tag=tag)
            nc.tensor.matmul(pv_ps[(h, qh)][0:DK + 1, :],
                             vhp[kt][:, h, :],
                             exp_tiles[h][kt][:, qh * 512:(qh + 1) * 512],
                             start=(kt == 0), stop=(kt == NST - 1))

        def norm(p, qh):
            """Normalize PV results of pair p, query-half qh, into oall2."""
            for i, h in enumerate((2 * p, 2 * p + 1)):
                op = pv_ps.pop((h, qh))
                sm = smp.tile([P, 512], DT_F, name=f"sm_{h}_{qh}", tag="sm")
                nc.vector.reciprocal(sm[DK:DK + 1, :], op[DK:DK + 1, :])
                bn = bncp.tile([1, 512], DT_F, name=f"bn_{h}_{qh}", tag="bn")
                nc.sync.dma_start(out=bn[:, :], in_=sm[DK:DK + 1, :])
                bc = bcp.tile([DK, 512], DT_F, name=f"bc_{h}_{qh}", tag="bc")
                nc.sync.dma_start(out=bc[:, :], in_=bcast_ap(bn[0:1, :], DK))
                if i == 0:
                    nc.vector.tensor_mul(
                        oall2[p][0:DK, qh * 512:(qh + 1) * 512],
                        op[0:DK, :], bc[:, :])
                else:
                    ot = otp.tile([DK, 512], DT_B, name=f"ot_{h}_{qh}", tag="ot")
                    nc.vector.tensor_mul(ot[:, :], op[0:DK, :], bc[:, :])
                    nc.sync.dma_start(
                        out=oall2[p][DK:P, qh * 512:(qh + 1) * 512],
                        in_=ot[:, :])

        # ---------------- emission schedule ----------------
        k_unit(0)
        q_unit(0)
        k_unit(1)
        q_unit(1)

        for p in range(NHP):
            for kt in range(NST):
                scores_exp(p, kt)
                if p >= 1:
                    # inline qh0 chains (pair 0's run as a catch-up burst)
                    pv_chain_mm(2 * p, 0, kt, ppv)
                    pv_chain_mm(2 * p + 1, 0, kt, ppv)
                if p == 0 and kt % 2 == 1:
                    v_unit(kt // 2)
                    if kt == NST - 1:
                        v_gather()
                if p >= 1 and p + 1 < NHP:
                    if kt == 2:
                        k_unit(p + 1)
                    if kt == 8:
                        q_unit(p + 1)
            if p == 0:
                vinp.release()
                # vhp is arriving: run pair 0's qh0 chains as a burst
                for kt in range(NST):
                    pv_chain_mm(0, 0, kt, ppv)
                    pv_chain_mm(1, 0, kt, ppv)
            # deferred qh1 chains for pair p on the aux banks
            for kt in range(NST):
                pv_chain_mm(2 * p, 1, kt, paux)
                pv_chain_mm(2 * p + 1, 1, kt, paux)
            norm(p, 0)
            norm(p, 1)
            if p == 6:
                qinp.release()
                kinp.release()

        # ---------------- phase C: output projection ----------------
        ystp = es.enter_context(tc.tile_pool(name="ystp", bufs=4))
        pwsb = [ystp.tile([P, DIM], DT_B, name=f"pwsb{hp}", bufs=1)
                for hp in range(NHP)]
        for hp in range(NHP):
            nc.sync.dma_start(out=pwsb[hp][:, :], in_=pw[hp * P:(hp + 1) * P, :])
        for dt_ in range(NDCH):
            ps = [pse.tile([P, 512], DT_F, name=f"yps_{dt_}_0", tag="se"),
                  pso.tile([P, 512], DT_F, name=f"yps_{dt_}_1", tag="so")]
            for hp in range(NHP):
                for n in range(2):
                    nc.tensor.matmul(ps[n][:, :],
                                     pwsb[hp][:, dt_ * P:(dt_ + 1) * P],
                                     oall2[hp][:, n * 512:(n + 1) * 512],
                                     start=(hp == 0), stop=(hp == NHP - 1))
            for n in range(2):
                yst = ystp.tile([P, 512], DT_F, name=f"yst_{dt_}_{n}", tag="yst")
                nc.vector.tensor_scalar_add(yst[:, :], ps[n][:, :],
                                            pbt[:, dt_:dt_ + 1])
                nc.sync.dma_start(
                    out=yT[dt_ * P:(dt_ + 1) * P, n * 512:(n + 1) * 512],
                    in_=yst[:, :])

    nc.compile()
    return nc


def kernel(q, k, v, w_q, w_k, w_v, proj_w, proj_b):
    global _NC, LAST_RESULT
    import ml_dtypes
    from concourse.bass_utils import run_bass_kernel_spmd

    if _NC is None:
        _NC = _build()

    bf16 = ml_dtypes.bfloat16
    q = np.asarray(q, dtype=np.float32)
    k = np.asarray(k, dtype=np.float32)
    v = np.asarray(v, dtype=np.float32)
    w_q = np.asarray(w_q, dtype=np.float32)
    w_k = np.asarray(w_k, dtype=np.float32)
    w_v = np.asarray(w_v, dtype=np.float32)
    proj_w = np.asarray(proj_w, dtype=np.float32)
    proj_b = np.asarray(proj_b, dtype=np.float32)

    wq2 = np.ascontiguousarray(
        np.transpose(w_q, (1, 0, 2)).reshape(DIM, H * DK)).astype(bf16)
    wk2 = np.ascontiguousarray(
        np.transpose(w_k, (1, 0, 2)).reshape(DIM, H * DK)).astype(bf16)
    wv2 = np.ascontiguousarray(
        np.transpose(w_v, (1, 0, 2)).reshape(DIM, H * DK)).astype(bf16)
    pwT = np.ascontiguousarray(proj_w.T).astype(bf16)
    pb2 = np.ascontiguousarray(proj_b.reshape(NDCH, P).T)

    in_maps = []
    for c in range(N_CORES):
        b, qo = c // 2, c % 2
        in_maps.append({
            "qT": np.ascontiguousarray(
                q[b, qo * TQ:(qo + 1) * TQ, :].T).astype(bf16),
            "kTh": np.ascontiguousarray(
                k[b, qo * TQ:(qo + 1) * TQ, :].T).astype(bf16),
            "vTh": np.ascontiguousarray(
                v[b, qo * TQ:(qo + 1) * TQ, :].T).astype(bf16),
            "wq": wq2, "wk": wk2, "wv": wv2,
            "pwT": pwT, "pb": pb2,
        })

    res = run_bass_kernel_spmd(_NC, in_maps, list(range(N_CORES)), trace=TRACE)
    LAST_RESULT = res

    out = np.empty((B, L, DIM), dtype=np.float32)
    for c in range(N_CORES):
        b, qo = c // 2, c % 2
        out[b, qo * TQ:(qo + 1) * TQ, :] = res.results[c]["yT"].T
    return out


# revision 8
# speedup vs baseline: 1.0777x; 1.0777x over previous
"""Multi-head attention kernel for 8 Trainium2 NeuronCores — v3.

Problem: B=4, L=2048, DIM=1024, H=16 heads, d_k=d_v=64.
Sharding: data-parallel over (batch, query-half); K/V projected half per
core, merged with a core-pair AllGather.

Design:
- Scores are row-tiled K=64 matmuls: the head pair's (even, odd)
  stationaries live at PE rows 0:64 / 64:128 and run concurrently.
- The scalar engine's exp (256 x [128,1024] activations, ~293us) is the
  bottleneck; all other work hides in its shadow. Per attention pair:
  scores (PE) -> exp (ACT) -> 4 inline PV chains (e/o x query-half,
  PSUM "pv" pool, 4 banks), with the K/Q projection units for pair p+1
  threaded through the same pv-pool rotation between pairs. V
  projection units are woven through pair 0 (its PV chains start late
  and catch up), with the AllGather split in two so vhp tiles arrive
  early.
- PSUM: scores_e 2 + scores_o 2 + pv 4 = 8 banks.
- Dense output projection: oall2[hp] is [128 = head pair, 1024 q]; the
  odd head's normalized result is DMA partition-shifted to rows 64:128.
- Softmax denominators ride as a ones-column in the PV stationary
  (M=65); exact DVE reciprocal; partition-broadcast via a DRAM bounce.
"""

import numpy as np

P = 128
B, L, DIM, H, DK = 4, 2048, 1024, 16, 64
TQ = 1024      # q tokens per core
TS = 2048      # kv tokens per core
NDCH = DIM // P          # 8 contraction chunks
NHP = H // 2             # 8 head pairs
NST = TS // P            # 16 key tiles
N_CORES = 8

_NC = None
TRACE = False
LAST_RESULT = None


def _build():
    from contextlib import ExitStack

    import concourse.bass as bass
    from concourse import bacc
    import concourse.mybir as mybir
    import concourse.tile as tile

    DT_B = mybir.dt.bfloat16
    DT_F = mybir.dt.float32
    AF = mybir.ActivationFunctionType

    nc = bacc.Bacc(None, target_bir_lowering=False)
    qT = nc.dram_tensor("qT", [DIM, TQ], DT_B, kind="ExternalInput")
    kT = nc.dram_tensor("kTh", [DIM, TS // 2], DT_B, kind="ExternalInput")
    vT = nc.dram_tensor("vTh", [DIM, TS // 2], DT_B, kind="ExternalInput")
    wq = nc.dram_tensor("wq", [DIM, H * DK], DT_B, kind="ExternalInput")
    wk = nc.dram_tensor("wk", [DIM, H * DK], DT_B, kind="ExternalInput")
    wv = nc.dram_tensor("wv", [DIM, H * DK], DT_B, kind="ExternalInput")
    pw = nc.dram_tensor("pwT", [H * DK, DIM], DT_B, kind="ExternalInput")
    pb = nc.dram_tensor("pb", [P, NDCH], DT_F, kind="ExternalInput")
    yT = nc.dram_tensor("yT", [DIM, TQ], DT_F, kind="ExternalOutput")

    def bcast_ap(ap, count):
        return bass.AP(tensor=ap.tensor, offset=ap.offset,
                       ap=[[0, count]] + [list(x) for x in ap.ap[1:]])

    with tile.TileContext(nc) as tc, ExitStack() as es:
        # ---------------- static SBUF pools ----------------
        l1 = es.enter_context(tc.tile_pool(name="l1", bufs=1))
        expp = es.enter_context(tc.tile_pool(name="expp", bufs=26))
        khtp = es.enter_context(tc.tile_pool(name="khtp", bufs=2))
        qhtp = es.enter_context(tc.tile_pool(name="qhtp", bufs=2))
        wqp = es.enter_context(tc.tile_pool(name="wqp", bufs=2))
        wkp = es.enter_context(tc.tile_pool(name="wkp", bufs=2))
        stg = es.enter_context(tc.tile_pool(name="stg", bufs=2))
        smp = es.enter_context(tc.tile_pool(name="smp", bufs=2))
        bcp = es.enter_context(tc.tile_pool(name="bcp", bufs=2))
        otp = es.enter_context(tc.tile_pool(name="otp", bufs=2))
        # ---------------- PSUM pools: 2+2+4 = 8 banks ----------------
        pse = es.enter_context(tc.tile_pool(name="pse", bufs=1, space="PSUM"))
        pso = es.enter_context(tc.tile_pool(name="pso", bufs=1, space="PSUM"))
        ppv = es.enter_context(tc.tile_pool(name="ppv", bufs=4, space="PSUM"))
        # ---------------- DRAM pools ----------------
        drp = es.enter_context(tc.tile_pool(name="drp", bufs=1, space="DRAM"))
        bncp = es.enter_context(tc.tile_pool(name="bncp", bufs=8, space="DRAM"))
        # input pools, released mid-emission once consumed (LIFO order)
        kinp = tc.alloc_tile_pool(name="kinp", bufs=1)
        qinp = tc.alloc_tile_pool(name="qinp", bufs=1)
        vinp = tc.alloc_tile_pool(name="vinp", bufs=1)

        # ---------------- whole-kernel tiles ----------------
        vhp = [l1.tile([P, H, DK + 1], DT_B, name=f"vhp{i}") for i in range(NST)]
        oall2 = [l1.tile([P, TQ], DT_B, name=f"oall2_{i}") for i in range(NHP)]
        pbt = l1.tile([P, NDCH], DT_F, name="pbt")
        nc.sync.dma_start(out=pbt[:, :], in_=pb[:, :])

        # ---------------- input DMAs ----------------
        vin = [vinp.tile([P, TS // 2], DT_B, name=f"vin{d}") for d in range(NDCH)]
        wvt = [vinp.tile([P, H * DK], DT_B, name=f"wvt{d}") for d in range(NDCH)]
        kin = [kinp.tile([P, TS // 2], DT_B, name=f"kin{d}") for d in range(NDCH)]
        qin = [qinp.tile([P, TQ], DT_B, name=f"qin{d}") for d in range(NDCH)]
        for d in range(NDCH):
            sl = slice(d * P, (d + 1) * P)
            nc.scalar.dma_start(out=vin[d][:, :], in_=vT[sl, :])
            nc.scalar.dma_start(out=wvt[d][:, :], in_=wv[sl, :])
            nc.sync.dma_start(out=kin[d][:, :], in_=kT[sl, :])
            nc.gpsimd.dma_start(out=qin[d][:, :], in_=qT[sl, :])

        # DRAM staging for collectives
        kout = [drp.tile([P, TS // 2], DT_B, name=f"kout{hp}")
                for hp in range(NHP)]
        kgath = [drp.tile([2, P, TS // 2], DT_B, name=f"kgath{hp}")
                 for hp in range(NHP)]
        vout = [drp.tile([4, P, H * (DK + 1)], DT_B, name=f"vout{g}")
                for g in range(2)]
        vgath = [drp.tile([2, 4, P, H * (DK + 1)], DT_B, name=f"vgath{g}")
                 for g in range(2)]

        kht = [None] * NHP
        qht = [None] * NHP

        # ---------------- projection units ----------------
        def k_unit(hp):
            """Project this core's key-half for head pair hp, AllGather."""
            wsl = wkp.tile([P, NDCH, P], DT_B, name=f"wksl{hp}", tag="wk")
            nc.sync.dma_start(
                out=wsl[:, :, :],
                in_=wk[:, hp * P:(hp + 1) * P].rearrange("(c p) n -> p c n", p=P))
            ps = [ppv.tile([P, 512], DT_F, name=f"kps_{hp}_{n}", tag="pv")
                  for n in range(2)]
            for d in range(NDCH):
                for n in range(2):
                    nc.tensor.matmul(ps[n][:, :],
                                     wsl[:, d, :],
                                     kin[d][:, n * 512:(n + 1) * 512],
                                     start=(d == 0), stop=(d == NDCH - 1))
            kst = stg.tile([P, TS // 2], DT_B, name=f"kst{hp}", tag="kst")
            for n in range(2):
                nc.vector.tensor_copy(kst[:, n * 512:(n + 1) * 512], ps[n][:, :])
            nc.sync.dma_start(out=kout[hp][:, :], in_=kst[:, :])
            nc.gpsimd.collective_compute(
                "AllGather", mybir.AluOpType.bypass,
                replica_groups=[[0, 1], [2, 3], [4, 5], [6, 7]],
                ins=[kout[hp][:, :]], outs=[kgath[hp][:, :, :]])
            kht[hp] = khtp.tile([P, TS], DT_B, name=f"kht{hp}", tag="kht")
            for half in range(2):
                nc.gpsimd.dma_start(
                    out=kht[hp][:, half * (TS // 2):(half + 1) * (TS // 2)],
                    in_=kgath[hp][half, :, :])

        def q_unit(hp):
            wsl = wqp.tile([P, NDCH, P], DT_B, name=f"wqsl{hp}", tag="wq")
            nc.sync.dma_start(
                out=wsl[:, :, :],
                in_=wq[:, hp * P:(hp + 1) * P].rearrange("(c p) n -> p c n", p=P))
            ps = [ppv.tile([P, 512], DT_F, name=f"qps_{hp}_{n}", tag="pv")
                  for n in range(2)]
            for d in range(NDCH):
                for n in range(2):
                    nc.tensor.matmul(ps[n][:, :],
                                     wsl[:, d, :],
                                     qin[d][:, n * 512:(n + 1) * 512],
                                     start=(d == 0), stop=(d == NDCH - 1))
            qht[hp] = qhtp.tile([P, TQ], DT_B, name=f"qht{hp}", tag="qht")
            for n in range(2):
                nc.vector.tensor_copy(qht[hp][:, n * 512:(n + 1) * 512],
                                      ps[n][:, :])

        def v_unit(st):
            """Project this core's value-half for local key tile st (of 8)."""
            g, j = st // 4, st % 4
            ps = [ppv.tile([P, 512], DT_F, name=f"vps_{st}_{n}", tag="pv")
                  for n in range(2)]
            for d in range(NDCH):
                for n in range(2):
                    nc.tensor.matmul(ps[n][:, :],
                                     vin[d][:, st * P:(st + 1) * P],
                                     wvt[d][:, n * 512:(n + 1) * 512],
                                     start=(d == 0), stop=(d == NDCH - 1))
            vst = stg.tile([P, H, DK + 1], DT_B, name=f"vst{st}", tag="vst")
            for n in range(2):
                nc.vector.tensor_copy(
                    vst[:, n * 8:(n + 1) * 8, 0:DK],
                    ps[n][:, :].rearrange("p (h d) -> p h d", d=DK))
            nc.vector.memset(vst[:, :, DK:DK + 1], 1.0)
            nc.sync.dma_start(out=vout[g][j, :, :],
                              in_=vst[:, :, :].rearrange("p h d -> p (h d)"))

        def v_gather(g):
            """AllGather V group g (local tiles 4g..4g+3 -> global
            vhp[8*half + 4g .. +3] for both halves)."""
            nc.gpsimd.collective_compute(
                "AllGather", mybir.AluOpType.bypass,
                replica_groups=[[0, 1], [2, 3], [4, 5], [6, 7]],
                ins=[vout[g][:, :, :]], outs=[vgath[g][:, :, :, :]])
            for half in range(2):
                for j in range(4):
                    nc.gpsimd.dma_start(
                        out=vhp[half * (NST // 2) + 4 * g + j][:, :, :],
                        in_=vgath[g][half, j, :, :].rearrange(
                            "p (h d) -> p h d", d=DK + 1))

        # ---------------- phase B pieces ----------------
        exp_tiles = [[None] * NST for _ in range(H)]

        def scores_exp(p, kt):
            sp_e = pse.tile([P, TQ], DT_F, name=f"se_{p}_{kt}", tag="se")
            sp_o = pso.tile([P, TQ], DT_F, name=f"so_{p}_{kt}", tag="so")
            for n in range(2):
                nc.tensor.matmul(
                    sp_e[:, n * 512:(n + 1) * 512],
                    kht[p][0:DK, kt * P:(kt + 1) * P],
                    qht[p][0:DK, n * 512:(n + 1) * 512],
                    start=True, stop=True)
                nc.tensor.matmul(
                    sp_o[:, n * 512:(n + 1) * 512],
                    kht[p][DK:P, kt * P:(kt + 1) * P],
                    qht[p][DK:P, n * 512:(n + 1) * 512],
                    start=True, stop=True)
            ex_e = expp.tile([P, TQ], DT_B, name=f"exe_{p}_{kt}", tag="exp")
            ex_o = expp.tile([P, TQ], DT_B, name=f"exo_{p}_{kt}", tag="exp")
            nc.scalar.activation(ex_e[:, :], sp_e[:, :], AF.Exp, scale=1.0 / 32.0)
            nc.scalar.activation(ex_o[:, :], sp_o[:, :], AF.Exp, scale=1.0 / 32.0)
            exp_tiles[2 * p][kt] = ex_e
            exp_tiles[2 * p + 1][kt] = ex_o

        pv_ps = {}   # (head, qh) -> psum tile of the running chain

        def pv_alloc(p):
            for h in (2 * p, 2 * p + 1):
                for qh in range(2):
                    pv_ps[(h, qh)] = ppv.tile([P, 512], DT_F,
                                              name=f"pv_{h}_{qh}", tag="pv")

        def pv_mm(p, kt):
            for h in (2 * p, 2 * p + 1):
                for qh in range(2):
                    nc.tensor.matmul(
                        pv_ps[(h, qh)][0:DK + 1, :],
                        vhp[kt][:, h, :],
                        exp_tiles[h][kt][:, qh * 512:(qh + 1) * 512],
                        start=(kt == 0), stop=(kt == NST - 1))

        def norm(p, qh):
            """Normalize PV results of pair p, query-half qh, into oall2."""
            for i, h in enumerate((2 * p, 2 * p + 1)):
                op = pv_ps.pop((h, qh))
                sm = smp.tile([P, 512], DT_F, name=f"sm_{h}_{qh}", tag="sm")
                nc.vector.reciprocal(sm[DK:DK + 1, :], op[DK:DK + 1, :])
                bn = bncp.tile([1, 512], DT_F, name=f"bn_{h}_{qh}", tag="bn")
                nc.sync.dma_start(out=bn[:, :], in_=sm[DK:DK + 1, :])
                bc = bcp.tile([DK, 512], DT_F, name=f"bc_{h}_{qh}", tag="bc")
                nc.sync.dma_start(out=bc[:, :], in_=bcast_ap(bn[0:1, :], DK))
                if i == 0:
                    nc.vector.tensor_mul(
                        oall2[p][0:DK, qh * 512:(qh + 1) * 512],
                        op[0:DK, :], bc[:, :])
                else:
                    ot = otp.tile([DK, 512], DT_B, name=f"ot_{h}_{qh}", tag="ot")
                    nc.vector.tensor_mul(ot[:, :], op[0:DK, :], bc[:, :])
                    nc.sync.dma_start(
                        out=oall2[p][DK:P, qh * 512:(qh + 1) * 512],
                        in_=ot[:, :])

        # ---------------- emission schedule ----------------
        k_unit(0)
        q_unit(0)
        k_unit(1)
        q_unit(1)

        # V projection units woven through pair 0; AllGather in 2 groups.
        V_SLOTS = {0: (0,), 2: (1,), 3: (2,), 5: (3,), 6: (4,), 8: (5,),
                   9: (6,), 11: (7,)}

        backlog = []   # (p, kt) PV mm groups not yet emitted

        for p in range(NHP):
            if p >= 1:
                if p + 1 < NHP:
                    k_unit(p + 1)
                    q_unit(p + 1)
                pv_alloc(p)
            for kt in range(NST):
                scores_exp(p, kt)
                if p == 0:
                    for u in V_SLOTS.get(kt, ()):
                        v_unit(u)
                        if u == 3:
                            v_gather(0)
                        elif u == 7:
                            v_gather(1)
                            pv_alloc(0)
                    if kt >= 12:
                        # drain pair-0 backlog aggressively once vhp exists
                        backlog.append((p, kt))
                        for _ in range(4):
                            if backlog:
                                pv_mm(*backlog.pop(0))
                    else:
                        backlog.append((p, kt))
                else:
                    backlog.append((p, kt))
                    n_drain = 2 if backlog else 0
                    for _ in range(n_drain):
                        if backlog:
                            pv_mm(*backlog.pop(0))
            if p == 0:
                vinp.release()
            while backlog:
                pv_mm(*backlog.pop(0))
            norm(p, 0)
            norm(p, 1)
            if p == NHP - 2:
                qinp.release()
                kinp.release()

        # ---------------- phase C: output projection ----------------
        ystp = es.enter_context(tc.tile_pool(name="ystp", bufs=4))
        pwsb = [ystp.tile([P, DIM], DT_B, name=f"pwsb{hp}", bufs=1)
                for hp in range(NHP)]
        for hp in range(NHP):
            nc.sync.dma_start(out=pwsb[hp][:, :], in_=pw[hp * P:(hp + 1) * P, :])
        for dt_ in range(NDCH):
            ps = [pse.tile([P, 512], DT_F, name=f"yps_{dt_}_0", tag="se"),
                  pso.tile([P, 512], DT_F, name=f"yps_{dt_}_1", tag="so")]
            for hp in range(NHP):
                for n in range(2):
                    nc.tensor.matmul(ps[n][:, :],
                                     pwsb[hp][:, dt_ * P:(dt_ + 1) * P],
                                     oall2[hp][:, n * 512:(n + 1) * 512],
                                     start=(hp == 0), stop=(hp == NHP - 1))
            for n in range(2):
                yst = ystp.tile([P, 512], DT_F, name=f"yst_{dt_}_{n}", tag="yst")
                nc.vector.tensor_scalar_add(yst[:, :], ps[n][:, :],
                                            pbt[:, dt_:dt_ + 1])
                nc.sync.dma_start(
                    out=yT[dt_ * P:(dt_ + 1) * P, n * 512:(n + 1) * 512],
                    in_=yst[:, :])

    nc.compile()
    return nc


def kernel(q, k, v, w_q, w_k, w_v, proj_w, proj_b):
    global _NC, LAST_RESULT
    import ml_dtypes
    from concourse.bass_utils import run_bass_kernel_spmd

    if _NC is None:
        _NC = _build()

    bf16 = ml_dtypes.bfloat16
    q = np.asarray(q, dtype=np.float32)
    k = np.asarray(k, dtype=np.float32)
    v = np.asarray(v, dtype=np.float32)
    w_q = np.asarray(w_q, dtype=np.float32)
    w_k = np.asarray(w_k, dtype=np.float32)
    w_v = np.asarray(w_v, dtype=np.float32)
    proj_w = np.asarray(proj_w, dtype=np.float32)
    proj_b = np.asarray(proj_b, dtype=np.float32)

    wq2 = np.ascontiguousarray(
        np.transpose(w_q, (1, 0, 2)).reshape(DIM, H * DK)).astype(bf16)
    wk2 = np.ascontiguousarray(
        np.transpose(w_k, (1, 0, 2)).reshape(DIM, H * DK)).astype(bf16)
    wv2 = np.ascontiguousarray(
        np.transpose(w_v, (1, 0, 2)).reshape(DIM, H * DK)).astype(bf16)
    pwT = np.ascontiguousarray(proj_w.T).astype(bf16)
    pb2 = np.ascontiguousarray(proj_b.reshape(NDCH, P).T)

    in_maps = []
    for c in range(N_CORES):
        b, qo = c // 2, c % 2
        in_maps.append({
            "qT": np.ascontiguousarray(
                q[b, qo * TQ:(qo + 1) * TQ, :].T).astype(bf16),
            "kTh": np.ascontiguousarray(
                k[b, qo * TQ:(qo + 1) * TQ, :].T).astype(bf16),
            "vTh": np.ascontiguousarray(
                v[b, qo * TQ:(qo + 1) * TQ, :].T).astype(bf16),
            "wq": wq2, "wk": wk2, "wv": wv2,
            "pwT": pwT, "pb": pb2,
        })

    res = run_bass_kernel_spmd(_NC, in_maps, list(range(N_CORES)), trace=TRACE)
    LAST_RESULT = res

    out = np.empty((B, L, DIM), dtype=np.float32)
    for c in range(N_CORES):
        b, qo = c // 2, c % 2
        out[b, qo * TQ:(qo + 1) * TQ, :] = res.results[c]["yT"].T
    return out
